# revision 1
# baseline (speedup 1.0000x reference)
"""Trainium2 Bass kernel for BilinearInteraction.

out[b, p] = x[b, i_p, :] @ W[p] @ x[b, j_p, :]  for the 780 field pairs
(i, j), i < j, of F=40 fields (row-major triu order).

Architecture (8 NeuronCores, data-parallel over batch, B_loc=256):
  - "b-T" layout: stage-1 PE matmuls produce Y[(pair, e), b] in PSUM
    (pairs x e on partitions, batch on the free dim), so the final
    e-reduction runs on the PE as ones-mask matmuls (contraction over
    partitions), keeping the vector engine to a single multiply pass.
  - Tiles: one [128, 256] PSUM slice holds 2 pairs sharing one i-field:
    (i, 2t) and (i, 2t+1), matching xT chunk t (fields 2t / 2t+1 on the
    two partition halves). W is host-permuted (bf16) into per-tile
    contiguous lhsT blocks (zero blocks for invalid (i==2t, 2t) slots).
    4 tiles of one chunk share a 2-bank [128, 1024] PSUM group so one
    tensor_tensor covers 4 tiles (amortizes the DVE PSUM-access bubble).
  - Host pre-transposes x into the three layouts the kernel needs
    (xtc f32 / xtcb bf16 for the multiply, xtlo bf16 for stage-1 rhs),
    eliminating all on-device transposes.
  - stage 1: PE matmul Y = Wtile.T @ xT_i  (bf16, K=64, M=128, N=256).
  - stage 2: ACT evicts Y -> bf16 SBUF; DVE multiplies by xtcb chunk at
    the 2x packed rate -> z bf16 (leftover small groups multiply PSUM
    directly on DVE at 1x).
  - stage 3: PE ones-mask matmuls (K=128, M=32) accumulate 16 z-tiles
    into one PSUM bank = 32 output pair-rows (full fp32 accumulation).
    Col-group tiling is deliberately NOT used: tile_position col-groups
    interleaved with full-width matmuls corrupt nondeterministically on
    real TRN2 hardware (verified; CoreSim is clean).
  - ACT evicts each result bank; DMA to outT[bank*32 + row, b]; the
    host inverse-permutes pair rows and concatenates the batch shards.
"""

import numpy as np
import ml_dtypes

import concourse.bass as bass
import concourse.mybir as mybir
import concourse.tile as tile
from concourse import bacc
from concourse.bass_utils import run_bass_kernel_spmd

B, F, D = 2048, 40, 64
P = F * (F - 1) // 2  # 780
NCORES = 8
B_LOC = B // NCORES  # 256
F32 = mybir.dt.float32
BF16 = mybir.dt.bfloat16

NCHUNK = F // 2  # 20 xT chunks (2 fields each)
# tile list: (t, i) — pairs (i, 2t) [dummy if i==2t] and (i, 2t+1)
TILES = [(t, i) for t in range(NCHUNK) for i in range(2 * t + 1)]
NTILES = len(TILES)  # 400
REDUCE_COLTILE = False
TILES_PER_BANK = 64 if REDUCE_COLTILE else 16
BANK_ROWS = 128 if REDUCE_COLTILE else 32
NBANKS = (NTILES + TILES_PER_BANK - 1) // TILES_PER_BANK
OUT_ROWS = NBANKS * BANK_ROWS


GMAX = 4  # tiles per PSUM group (4 x 256 cols = 2 banks)
REDUCE_DELAY = 9  # groups of reduce-matmul lag (software pipelining)


def _build_groups():
    # pairs of same-chunk tiles sharing one [128,512] PSUM bank; split at
    # bank boundaries so both reduce slots land in the same bank-pass
    groups = []
    k = 0
    for t in range(NCHUNK):
        ilist = list(range(2 * t + 1))
        while ilist:
            take = min(GMAX, len(ilist), TILES_PER_BANK - (k % TILES_PER_BANK))
            groups.append((t, ilist[:take]))
            ilist = ilist[take:]
            k += take
    return groups


GROUPS = _build_groups()

WDMA_BATCH = 8  # stage-1 lhsT tiles per DMA


def host_prep(W: np.ndarray):
    """Build Wt3 [64, NTILES*128] f32, ONES [128, 512] bf16, PERM info."""
    # Wt2[d, p, e]
    Wt2 = np.ascontiguousarray(W.transpose(1, 0, 2))  # [64, 780, 64]
    pair_idx = -np.ones((F, F), dtype=np.int64)
    k = 0
    for i in range(F):
        for j in range(i + 1, F):
            pair_idx[i, j] = k
            k += 1
    Wt3 = np.zeros((D, NTILES * 128), dtype=np.float32)  # cast to bf16 at end
    # rows[k] = (origA or -1, origB) for tile k
    rows = []
    for k, (t, i) in enumerate(TILES):
        jA, jB = 2 * t, 2 * t + 1
        pA = pair_idx[i, jA] if i < jA else -1
        pB = pair_idx[i, jB]
        if pA >= 0:
            Wt3[:, k * 128 : k * 128 + 64] = Wt2[:, pA, :]
        Wt3[:, k * 128 + 64 : k * 128 + 128] = Wt2[:, pB, :]
        rows.append((pA, pB))
    # ones masks: ONES[:, q*32+m] — slot q (0..15): col 2q active for k<64,
    # col 2q+1 active for k>=64
    ones = np.zeros((128, 512), dtype=np.float32)
    for q in range(16):
        ones[0:64, q * 32 + 2 * q] = 1.0
        ones[64:128, q * 32 + 2 * q + 1] = 1.0
    ones = ones.astype(ml_dtypes.bfloat16)
    # out row of tile k: bank = k//64, s = k%64, g = s%4, q = s//4
    # rowA = bank*128 + 32*g + 2*q ; rowB = rowA + 1
    perm_src = np.zeros(P, dtype=np.int64)  # outT row for original pair p
    for k, (pA, pB) in enumerate(rows):
        bank, s = divmod(k, TILES_PER_BANK)
        if REDUCE_COLTILE:
            g, q = s % 4, s // 4
            rowA = bank * BANK_ROWS + 32 * g + 2 * q
        else:
            rowA = bank * BANK_ROWS + 2 * s
        if pA >= 0:
            perm_src[pA] = rowA
        perm_src[pB] = rowA + 1
    return Wt3.astype(ml_dtypes.bfloat16), ones, perm_src


def build_nc():
    nc = bacc.Bacc("TRN2", target_bir_lowering=False, debug=False)

    xtc_dram = nc.dram_tensor(
        "xtc", [128, NCHUNK * B_LOC], F32, kind="ExternalInput"
    ).ap()
    xtlo_dram = nc.dram_tensor(
        "xtlo", [64, F * B_LOC], BF16, kind="ExternalInput"
    ).ap()
    xtcb_dram = nc.dram_tensor(
        "xtcb", [128, NCHUNK * B_LOC], BF16, kind="ExternalInput"
    ).ap()
    wt_dram = nc.dram_tensor("Wt3", [D, NTILES * 128], BF16, kind="ExternalInput").ap()
    ones_dram = nc.dram_tensor("ones", [128, 512], BF16, kind="ExternalInput").ap()
    out_dram = nc.dram_tensor("outT", [OUT_ROWS, B_LOC], F32, kind="ExternalOutput").ap()

    with tile.TileContext(nc) as tc:
        with (
            tc.tile_pool(name="persist", bufs=1) as persist,
            tc.tile_pool(name="wpool", bufs=4) as wpool,
            tc.tile_pool(name="zpool", bufs=14) as zpool,
            tc.tile_pool(name="ybfpool", bufs=6) as ybfpool,
            tc.tile_pool(name="opool", bufs=2) as opool,
            tc.tile_pool(name="ypsum", bufs=3, space=bass.MemorySpace.PSUM) as ypsum,
            tc.tile_pool(name="rpsum", bufs=2, space=bass.MemorySpace.PSUM) as rpsum,
        ):
            ones = persist.tile([128, 512], BF16, tag="ones")
            nc.sync.dma_start(out=ones[:], in_=ones_dram[:])

            # XTC[(f%2)*64 + d, t*256 + m*128 + b]  (f = 2t + f%2) and the
            # low-half layout (all fields at partitions 0-63) are both
            # pre-transposed on the host and DMA'd directly.
            xtc = persist.tile([128, NCHUNK * B_LOC], F32, tag="xtc")
            xtlo = persist.tile([64, F * B_LOC], BF16, tag="xtlo")
            xtcb = persist.tile([128, NCHUNK * B_LOC], BF16, tag="xtcb")
            nq = NCHUNK * B_LOC // 4
            nf = F * B_LOC // 4
            for c4 in range(4):
                nc.sync.dma_start(
                    out=xtc[:, c4 * nq : (c4 + 1) * nq],
                    in_=xtc_dram[:, c4 * nq : (c4 + 1) * nq],
                )
                nc.sync.dma_start(
                    out=xtlo[:, c4 * nf : (c4 + 1) * nf],
                    in_=xtlo_dram[:, c4 * nf : (c4 + 1) * nf],
                )
                nc.sync.dma_start(
                    out=xtcb[:, c4 * nq : (c4 + 1) * nq],
                    in_=xtcb_dram[:, c4 * nq : (c4 + 1) * nq],
                )

            rbs = [None]
            wchunk = None
            k = 0
            nquad = 0
            pending = []

            def emit_reduce(z, k0, gsz):
                # accumulate into reduce bank via ones-mask matmuls
                for idx in range(gsz):
                    kt = k0 + idx
                    bank, s = divmod(kt, TILES_PER_BANK)
                    q = s
                    if s == 0:
                        rbs[0] = rpsum.tile([128, B_LOC], F32, tag="rb", name="rb")
                    rb = rbs[0]
                    last_in_bank = (s == TILES_PER_BANK - 1) or (kt == NTILES - 1)
                    nc.tensor.matmul(
                        rb[0:32, :],
                        ones[:, q * 32 : (q + 1) * 32],
                        z[:, idx * B_LOC : (idx + 1) * B_LOC],
                        start=(s == 0),
                        stop=last_in_bank,
                        tile_position=(0, 0),
                        skip_group_check=True,
                    )
                    if last_in_bank:
                        ob = opool.tile([BANK_ROWS, B_LOC], F32, tag="ob")
                        nc.vector.tensor_copy(out=ob[:], in_=rb[0:BANK_ROWS, :])
                        nc.sync.dma_start(
                            out=out_dram[
                                bank * BANK_ROWS : (bank + 1) * BANK_ROWS, :
                            ],
                            in_=ob[:],
                        )

            for t, ilist in GROUPS:
                gsz = len(ilist)
                # stage 1: Y[(p, e), b] = Wtile.T @ xT_i — gsz tiles share
                # one PSUM bank (disjoint column halves)
                y = ypsum.tile([128, GMAX * B_LOC], F32, tag="y")
                for idx, i in enumerate(ilist):
                    kt = k + idx
                    if kt % WDMA_BATCH == 0:
                        nw = min(WDMA_BATCH, NTILES - kt)
                        wchunk = wpool.tile([64, WDMA_BATCH * 128], BF16, tag="w")
                        nc.sync.dma_start(
                            out=wchunk[:, : nw * 128],
                            in_=wt_dram[:, kt * 128 : (kt + nw) * 128],
                        )
                    kk = kt % WDMA_BATCH
                    nc.tensor.matmul(
                        y[:, idx * B_LOC : (idx + 1) * B_LOC],
                        wchunk[:, kk * 128 : (kk + 1) * 128],
                        xtlo[:, i * B_LOC : (i + 1) * B_LOC],
                        start=True,
                        stop=True,
                    )

                # stage 2: z = Y * xT[j-fields chunk t]  (bf16 out), one TT
                # per group with stride-0 broadcast of the xtc chunk.
                # A fraction of pair-groups takes the ACT-evict + GPSIMD
                # multiply path to unload the DVE.
                z = zpool.tile([128, GMAX * B_LOC], BF16, tag="z")
                if gsz > 1:
                    nquad += 1
                    act_path = gsz == GMAX
                    if act_path:
                        # ACT evicts PSUM -> bf16, DVE multiplies at 2x
                        ybf = ybfpool.tile([128, GMAX * B_LOC], BF16, tag="ybf")
                        if nquad % 9 == 1:
                            # spill ~1/18 of the PSUM drains to the DVE to
                            # equalize ACT/DVE/PE busy time
                            nc.vector.tensor_copy(out=ybf[:], in_=y[:])
                        else:
                            nc.scalar.copy(out=ybf[:], in_=y[:])
                        in1 = xtcb[
                            :, None, t * B_LOC : (t + 1) * B_LOC
                        ].to_broadcast([128, gsz, B_LOC])
                        nc.vector.tensor_tensor(
                            z[:, : gsz * B_LOC].rearrange(
                                "p (n b) -> p n b", n=gsz
                            ),
                            ybf[:, : gsz * B_LOC].rearrange(
                                "p (n b) -> p n b", n=gsz
                            ),
                            in1,
                            mybir.AluOpType.mult,
                        )
                    else:
                        in1 = xtc[
                            :, None, t * B_LOC : (t + 1) * B_LOC
                        ].to_broadcast([128, gsz, B_LOC])
                        nc.vector.tensor_tensor(
                            z[:, : gsz * B_LOC].rearrange(
                                "p (n b) -> p n b", n=gsz
                            ),
                            y[:, : gsz * B_LOC].rearrange(
                                "p (n b) -> p n b", n=gsz
                            ),
                            in1,
                            mybir.AluOpType.mult,
                        )
                else:
                    nc.vector.tensor_tensor(
                        z[:, :B_LOC],
                        y[:, :B_LOC],
                        xtc[:, t * B_LOC : (t + 1) * B_LOC],
                        mybir.AluOpType.mult,
                    )

                # stage 3 is software-pipelined: queue this group's reduce
                # and emit the one from REDUCE_DELAY groups ago, so the PE's
                # in-order queue never waits on the just-issued ACT->DVE
                # multiply chain.
                pending.append((z, k, gsz))
                if len(pending) > REDUCE_DELAY:
                    emit_reduce(*pending.pop(0))
                k += gsz

            while pending:
                emit_reduce(*pending.pop(0))

    nc.compile()
    return nc


_NC = None


def kernel(x: np.ndarray, W: np.ndarray) -> np.ndarray:
    global _NC
    x = np.ascontiguousarray(np.asarray(x, dtype=np.float32))
    W = np.ascontiguousarray(np.asarray(W, dtype=np.float32))
    assert x.shape == (B, F, D) and W.shape == (P, D, D)

    Wt3, ones, perm_src = host_prep(W)

    if _NC is None:
        _NC = build_nc()

    in_maps = []
    for c in range(NCORES):
        xs = x[c * B_LOC : (c + 1) * B_LOC]  # [256, 40, 64]
        v = xs.transpose(1, 2, 0).reshape(NCHUNK, 2, D, B_LOC)
        xtc = np.ascontiguousarray(
            v.transpose(1, 2, 0, 3).reshape(128, NCHUNK * B_LOC)
        )
        xtlo = np.ascontiguousarray(
            xs.transpose(2, 1, 0).reshape(D, F * B_LOC)
        ).astype(ml_dtypes.bfloat16)
        xtcb = xtc.astype(ml_dtypes.bfloat16)
        in_maps.append(
            {"xtc": xtc, "xtcb": xtcb, "xtlo": xtlo, "Wt3": Wt3, "ones": ones}
        )
    res = run_bass_kernel_spmd(_NC, in_maps, core_ids=list(range(NCORES)))
    out = np.empty((B, P), dtype=np.float32)
    for c in range(NCORES):
        outT = res.results[c]["outT"]  # [OUT_ROWS, B_LOC]
        out[c * B_LOC : (c + 1) * B_LOC, :] = outT[perm_src, :].T
    return out



# revision 15
# speedup vs baseline: 1.1030x; 1.1030x over previous
"""Trainium2 Bass kernel for BilinearInteraction.

out[b, p] = x[b, i_p, :] @ W[p] @ x[b, j_p, :]  for the 780 field pairs
(i, j), i < j, of F=40 fields (row-major triu order).

Architecture (8 NeuronCores, data-parallel over batch, B_loc=256):
  - "b-T" layout: stage-1 PE matmuls produce Y[(pair, e), b] in PSUM
    (pairs x e on partitions, batch on the free dim), so the final
    e-reduction runs on the PE as ones-mask matmuls (contraction over
    partitions), keeping the vector engine to a single multiply pass.
  - Tiles: one [128, 256] PSUM slice holds 2 pairs sharing one i-field:
    (i, 2t) and (i, 2t+1), matching xT chunk t (fields 2t / 2t+1 on the
    two partition halves). W is host-permuted (bf16) into per-tile
    contiguous lhsT blocks (zero blocks for invalid (i==2t, 2t) slots).
    4 tiles of one chunk share a 2-bank [128, 1024] PSUM group so one
    tensor_tensor covers 4 tiles (amortizes the DVE PSUM-access bubble).
  - Host pre-transposes x into the three layouts the kernel needs
    (xtc f32 / xtcb bf16 for the multiply, xtlo bf16 for stage-1 rhs),
    eliminating all on-device transposes.
  - stage 1: PE matmul Y = Wtile.T @ xT_i  (bf16, K=64, M=128, N=256).
  - stage 2: ACT evicts Y -> bf16 SBUF; DVE multiplies by xtcb chunk at
    the 2x packed rate -> z bf16 (leftover small groups multiply PSUM
    directly on DVE at 1x).
  - stage 3: PE ones-mask matmuls (K=128, M=32) accumulate 16 z-tiles
    into one PSUM bank = 32 output pair-rows (full fp32 accumulation).
    Col-group tiling is deliberately NOT used: tile_position col-groups
    interleaved with full-width matmuls corrupt nondeterministically on
    real TRN2 hardware (verified; CoreSim is clean).
  - ACT evicts each result bank; DMA to outT[bank*32 + row, b]; the
    host inverse-permutes pair rows and concatenates the batch shards.
"""

import numpy as np
import ml_dtypes

import concourse.bass as bass
import concourse.mybir as mybir
import concourse.tile as tile
from concourse import bacc
from concourse.bass_utils import run_bass_kernel_spmd

B, F, D = 2048, 40, 64
P = F * (F - 1) // 2  # 780
NCORES = 8
B_LOC = B // NCORES  # 256
F32 = mybir.dt.float32
BF16 = mybir.dt.bfloat16

NCHUNK = F // 2  # 20 xT chunks (2 fields each)
# tile list: (t, i) — pairs (i, 2t) [dummy if i==2t] and (i, 2t+1)
TILES = [(t, i) for t in range(NCHUNK) for i in range(2 * t + 1)]
NTILES = len(TILES)  # 400
REDUCE_COLTILE = False
TILES_PER_BANK = 64 if REDUCE_COLTILE else 16
BANK_ROWS = 128 if REDUCE_COLTILE else 32
NBANKS = (NTILES + TILES_PER_BANK - 1) // TILES_PER_BANK
OUT_ROWS = NBANKS * BANK_ROWS


GMAX = 4  # tiles per PSUM group (4 x 256 cols = 2 banks)
REDUCE_DELAY = 9  # groups of reduce-matmul lag (software pipelining)

# DMA prefetch slicing (units: xtlo = fields, xtc/xtcb = chunks). The first
# pieces are small so the first stage-1/mult tiles unblock early; x DMAs ride
# the ACT HWDGE queue so they overlap W DMAs issued on the SP queue.
XTLO_PIECES = [(0, 8), (8, 16), (16, 28), (28, 40)]
XCHUNK_PIECES = [(0, 4), (4, 8), (8, 14), (14, 20)]
WDMA_PREFETCH = 4  # W batches issued before the compute loop


def _build_groups():
    # pairs of same-chunk tiles sharing one [128,512] PSUM bank; split at
    # bank boundaries so both reduce slots land in the same bank-pass
    groups = []
    k = 0
    for t in range(NCHUNK):
        ilist = list(range(2 * t + 1))
        while ilist:
            take = min(GMAX, len(ilist), TILES_PER_BANK - (k % TILES_PER_BANK))
            groups.append((t, ilist[:take]))
            ilist = ilist[take:]
            k += take
    return groups


GROUPS = _build_groups()

WDMA_BATCH = 16  # stage-1 lhsT tiles per DMA


def host_prep(W: np.ndarray):
    """Build Wt3 [64, NTILES*128] f32, ONES [128, 512] bf16, PERM info."""
    # Wt2[d, p, e]
    Wt2 = np.ascontiguousarray(W.transpose(1, 0, 2))  # [64, 780, 64]
    pair_idx = -np.ones((F, F), dtype=np.int64)
    k = 0
    for i in range(F):
        for j in range(i + 1, F):
            pair_idx[i, j] = k
            k += 1
    Wt3 = np.zeros((D, NTILES * 128), dtype=np.float32)  # cast to bf16 at end
    # rows[k] = (origA or -1, origB) for tile k
    rows = []
    for k, (t, i) in enumerate(TILES):
        jA, jB = 2 * t, 2 * t + 1
        pA = pair_idx[i, jA] if i < jA else -1
        pB = pair_idx[i, jB]
        if pA >= 0:
            Wt3[:, k * 128 : k * 128 + 64] = Wt2[:, pA, :]
        Wt3[:, k * 128 + 64 : k * 128 + 128] = Wt2[:, pB, :]
        rows.append((pA, pB))
    # ones masks: ONES[:, q*32+m] — slot q (0..15): col 2q active for k<64,
    # col 2q+1 active for k>=64
    ones = np.zeros((128, 512), dtype=np.float32)
    for q in range(16):
        ones[0:64, q * 32 + 2 * q] = 1.0
        ones[64:128, q * 32 + 2 * q + 1] = 1.0
    ones = ones.astype(ml_dtypes.bfloat16)
    # out row of tile k: bank = k//64, s = k%64, g = s%4, q = s//4
    # rowA = bank*128 + 32*g + 2*q ; rowB = rowA + 1
    perm_src = np.zeros(P, dtype=np.int64)  # outT row for original pair p
    for k, (pA, pB) in enumerate(rows):
        bank, s = divmod(k, TILES_PER_BANK)
        if REDUCE_COLTILE:
            g, q = s % 4, s // 4
            rowA = bank * BANK_ROWS + 32 * g + 2 * q
        else:
            rowA = bank * BANK_ROWS + 2 * s
        if pA >= 0:
            perm_src[pA] = rowA
        perm_src[pB] = rowA + 1
    return Wt3.astype(ml_dtypes.bfloat16), ones, perm_src


def build_nc():
    nc = bacc.Bacc("TRN2", target_bir_lowering=False, debug=False)

    xtlo_dram = nc.dram_tensor(
        "xtlo", [64, F * B_LOC], BF16, kind="ExternalInput"
    ).ap()
    xtcb_dram = nc.dram_tensor(
        "xtcb", [128, NCHUNK * B_LOC], BF16, kind="ExternalInput"
    ).ap()
    wt_dram = nc.dram_tensor("Wt3", [D, NTILES * 128], BF16, kind="ExternalInput").ap()
    ones_dram = nc.dram_tensor("ones", [128, 512], BF16, kind="ExternalInput").ap()
    out_dram = nc.dram_tensor("outT", [OUT_ROWS, B_LOC], F32, kind="ExternalOutput").ap()

    with tile.TileContext(nc) as tc:
        with (
            tc.tile_pool(name="persist", bufs=1) as persist,
            tc.tile_pool(name="wpool", bufs=4) as wpool,
            tc.tile_pool(name="zpool", bufs=17) as zpool,
            tc.tile_pool(name="ybfpool", bufs=6) as ybfpool,
            tc.tile_pool(name="opool", bufs=2) as opool,
            tc.tile_pool(name="ypsum", bufs=3, space=bass.MemorySpace.PSUM) as ypsum,
            tc.tile_pool(name="rpsum", bufs=2, space=bass.MemorySpace.PSUM) as rpsum,
        ):
            ones = persist.tile([128, 512], BF16, tag="ones")

            # XTC[(f%2)*64 + d, t*256 + m*128 + b]  (f = 2t + f%2) and the
            # low-half layout (all fields at partitions 0-63) are both
            # pre-transposed on the host and DMA'd directly.
            xtlo = persist.tile([64, F * B_LOC], BF16, tag="xtlo")
            xtcb = persist.tile([128, NCHUNK * B_LOC], BF16, tag="xtcb")

            # W batches are DMA'd on the SP queue; x layouts ride the ACT
            # HWDGE queue so both descriptor-gen streams overlap. Pieces are
            # issued smallest-first so early tiles unblock within ~2us.
            wtiles = []

            def w_dma(bi):
                kt0 = bi * WDMA_BATCH
                nw = min(WDMA_BATCH, NTILES - kt0)
                wt = wpool.tile([64, WDMA_BATCH * 128], BF16, tag="w")
                nc.sync.dma_start(
                    out=wt[:, : nw * 128],
                    in_=wt_dram[:, kt0 * 128 : (kt0 + nw) * 128],
                )
                wtiles.append(wt)

            def x_dma(piece, eng):
                kind, lo, hi = piece
                src, dst = {
                    "xtlo": (xtlo_dram, xtlo),
                    "xtcb": (xtcb_dram, xtcb),
                }[kind]
                eng.dma_start(
                    out=dst[:, lo * B_LOC : hi * B_LOC],
                    in_=src[:, lo * B_LOC : hi * B_LOC],
                )

            # First pieces on the ACT HWDGE queue (3 gens before the first
            # evict dispatch); the tail rides Pool SWDGE, which is idle until
            # the out-DMAs begin.
            w_dma(0)
            x_dma(("xtlo",) + XTLO_PIECES[0], nc.scalar)
            x_dma(("xtcb",) + XCHUNK_PIECES[0], nc.scalar)
            w_dma(1)
            x_dma(("xtcb",) + XCHUNK_PIECES[1], nc.gpsimd)
            x_dma(("xtlo",) + XTLO_PIECES[1], nc.scalar)
            w_dma(2)
            x_dma(("xtcb",) + XCHUNK_PIECES[2], nc.gpsimd)
            x_dma(("xtlo",) + XTLO_PIECES[2], nc.gpsimd)
            w_dma(3)
            x_dma(("xtcb",) + XCHUNK_PIECES[3], nc.gpsimd)
            x_dma(("xtlo",) + XTLO_PIECES[3], nc.gpsimd)
            nc.sync.dma_start(out=ones[:], in_=ones_dram[:])

            rbs = [None]
            k = 0
            nquad = 0
            pending = []

            def emit_reduce(z, k0, gsz):
                # accumulate into reduce bank via ones-mask matmuls
                for idx in range(gsz):
                    kt = k0 + idx
                    bank, s = divmod(kt, TILES_PER_BANK)
                    q = s
                    if s == 0:
                        rbs[0] = rpsum.tile([128, B_LOC], F32, tag="rb", name="rb")
                    rb = rbs[0]
                    last_in_bank = (s == TILES_PER_BANK - 1) or (kt == NTILES - 1)
                    nc.tensor.matmul(
                        rb[0:32, :],
                        ones[:, q * 32 : (q + 1) * 32],
                        z[:, idx * B_LOC : (idx + 1) * B_LOC],
                        start=(s == 0),
                        stop=last_in_bank,
                        tile_position=(0, 0),
                        skip_group_check=True,
                    )
                    if last_in_bank:
                        ob = opool.tile([BANK_ROWS, B_LOC], F32, tag="ob")
                        # alternate the result-bank eviction between DVE and
                        # ACT so neither exceeds its steady-state budget
                        if bank % 2 == 0:
                            nc.vector.tensor_copy(out=ob[:], in_=rb[0:BANK_ROWS, :])
                        else:
                            nc.scalar.copy(out=ob[:], in_=rb[0:BANK_ROWS, :])
                        # SWDGE (Pool) out-DMA: keeps the SP queue free for W
                        # batches and dodges the shared HWDGE generator.
                        nc.gpsimd.dma_start(
                            out=out_dram[
                                bank * BANK_ROWS : (bank + 1) * BANK_ROWS, :
                            ],
                            in_=ob[:],
                        )

            for t, ilist in GROUPS:
                gsz = len(ilist)
                # stage 1: Y[(p, e), b] = Wtile.T @ xT_i — gsz tiles share
                # one PSUM bank (disjoint column halves)
                y = ypsum.tile([128, GMAX * B_LOC], F32, tag="y")
                for idx, i in enumerate(ilist):
                    kt = k + idx
                    if kt % WDMA_BATCH == 0:
                        bi = kt // WDMA_BATCH + 2  # two batches of DMA lead
                        if WDMA_PREFETCH <= bi < (NTILES + WDMA_BATCH - 1) // WDMA_BATCH:
                            w_dma(bi)
                    kk = kt % WDMA_BATCH
                    wchunk = wtiles[kt // WDMA_BATCH]
                    nc.tensor.matmul(
                        y[:, idx * B_LOC : (idx + 1) * B_LOC],
                        wchunk[:, kk * 128 : (kk + 1) * 128],
                        xtlo[:, i * B_LOC : (i + 1) * B_LOC],
                        start=True,
                        stop=True,
                    )

                # stage 2: z = Y * xT[j-fields chunk t]  (bf16 out), one TT
                # per group with stride-0 broadcast of the xtc chunk.
                # A fraction of pair-groups takes the ACT-evict + GPSIMD
                # multiply path to unload the DVE.
                z = zpool.tile([128, GMAX * B_LOC], BF16, tag="z")
                if gsz > 1:
                    nquad += 1
                    act_path = gsz == GMAX and nquad % 5 != 1
                    if act_path:
                        # ACT evicts PSUM -> bf16, DVE multiplies at 2x
                        ybf = ybfpool.tile([128, GMAX * B_LOC], BF16, tag="ybf")
                        nc.scalar.copy(out=ybf[:], in_=y[:])
                        in1 = xtcb[
                            :, None, t * B_LOC : (t + 1) * B_LOC
                        ].to_broadcast([128, gsz, B_LOC])
                        nc.vector.tensor_tensor(
                            z[:, : gsz * B_LOC].rearrange(
                                "p (n b) -> p n b", n=gsz
                            ),
                            ybf[:, : gsz * B_LOC].rearrange(
                                "p (n b) -> p n b", n=gsz
                            ),
                            in1,
                            mybir.AluOpType.mult,
                        )
                    else:
                        # every 9th quad drains fused on the DVE (PSUM f32
                        # multiply at 1x) to keep ACT under the PE roofline
                        in1 = xtcb[
                            :, None, t * B_LOC : (t + 1) * B_LOC
                        ].to_broadcast([128, gsz, B_LOC])
                        nc.vector.tensor_tensor(
                            z[:, : gsz * B_LOC].rearrange(
                                "p (n b) -> p n b", n=gsz
                            ),
                            y[:, : gsz * B_LOC].rearrange(
                                "p (n b) -> p n b", n=gsz
                            ),
                            in1,
                            mybir.AluOpType.mult,
                        )
                else:
                    nc.vector.tensor_tensor(
                        z[:, :B_LOC],
                        y[:, :B_LOC],
                        xtcb[:, t * B_LOC : (t + 1) * B_LOC],
                        mybir.AluOpType.mult,
                    )

                # stage 3 is software-pipelined: queue this group's reduce
                # and emit the one from REDUCE_DELAY groups ago, so the PE's
                # in-order queue never waits on the just-issued ACT->DVE
                # multiply chain.
                pending.append((z, k, gsz))
                if len(pending) > REDUCE_DELAY:
                    emit_reduce(*pending.pop(0))
                k += gsz

            while pending:
                emit_reduce(*pending.pop(0))

    nc.compile()
    return nc


_NC = None


def kernel(x: np.ndarray, W: np.ndarray) -> np.ndarray:
    global _NC
    x = np.ascontiguousarray(np.asarray(x, dtype=np.float32))
    W = np.ascontiguousarray(np.asarray(W, dtype=np.float32))
    assert x.shape == (B, F, D) and W.shape == (P, D, D)

    Wt3, ones, perm_src = host_prep(W)

    if _NC is None:
        _NC = build_nc()

    in_maps = []
    for c in range(NCORES):
        xs = x[c * B_LOC : (c + 1) * B_LOC]  # [256, 40, 64]
        v = xs.transpose(1, 2, 0).reshape(NCHUNK, 2, D, B_LOC)
        xtc = np.ascontiguousarray(
            v.transpose(1, 2, 0, 3).reshape(128, NCHUNK * B_LOC)
        )
        xtlo = np.ascontiguousarray(
            xs.transpose(2, 1, 0).reshape(D, F * B_LOC)
        ).astype(ml_dtypes.bfloat16)
        xtcb = xtc.astype(ml_dtypes.bfloat16)
        in_maps.append(
            {"xtcb": xtcb, "xtlo": xtlo, "Wt3": Wt3, "ones": ones}
        )
    res = run_bass_kernel_spmd(_NC, in_maps, core_ids=list(range(NCORES)))
    out = np.empty((B, P), dtype=np.float32)
    for c in range(NCORES):
        outT = res.results[c]["outT"]  # [OUT_ROWS, B_LOC]
        out[c * B_LOC : (c + 1) * B_LOC, :] = outT[perm_src, :].T
    return out



# revision 34
# speedup vs baseline: 1.1328x; 1.0270x over previous
"""Trainium2 Bass kernel for BilinearInteraction.

out[b, p] = x[b, i_p, :] @ W[p] @ x[b, j_p, :]  for the 780 field pairs
(i, j), i < j, of F=40 fields (row-major triu order).

Architecture (8 NeuronCores, data-parallel over batch, B_loc=256):
  - "b-T" layout: stage-1 PE matmuls produce Y[(pair, e), b] in PSUM
    (pairs x e on partitions, batch on the free dim), so the final
    e-reduction runs on the PE as ones-mask matmuls (contraction over
    partitions), keeping the vector engine to a single multiply pass.
  - Tiles: one [128, 256] PSUM slice holds 2 pairs sharing one i-field:
    (i, 2t) and (i, 2t+1), matching xT chunk t (fields 2t / 2t+1 on the
    two partition halves). W is host-permuted (bf16) into per-tile
    contiguous lhsT blocks (zero blocks for invalid (i==2t, 2t) slots).
    4 tiles of one chunk share a 2-bank [128, 1024] PSUM group so one
    tensor_tensor covers 4 tiles (amortizes the DVE PSUM-access bubble).
  - Host pre-transposes x into the two layouts the kernel needs
    (xtcb bf16 for the multiply, xtlo bf16 for stage-1 rhs),
    eliminating all on-device transposes.
  - DMA queues: W batches + ones on SP, x pieces split across the ACT
    HWDGE queue (first pieces) and Pool SWDGE (tail), out-DMAs on Pool
    SWDGE (last two on SP to shorten the drain tail). Pieces are
    ordered so the first stage-1/mult tiles unblock within ~3.5us.
  - stage 1: PE matmul Y = Wtile.T @ xT_i  (bf16, K=64, M=128, N=256).
  - stage 2: ACT evicts Y -> bf16 SBUF; DVE multiplies by xtcb chunk at
    the 2x packed rate -> z bf16. Every 4th quad (and the small
    leftover groups) instead drains fused on the DVE (PSUM f32 x bf16
    at 1x) because a pure ACT evict path (1038ns/quad) cannot keep up
    with the PE's 854ns/quad group cadence.
  - stage 3: PE ones-mask matmuls (K=128, M=32) accumulate 16 z-tiles
    into one PSUM bank = 32 output pair-rows (full fp32 accumulation).
    Col-group tiling is deliberately NOT used: tile_position col-groups
    interleaved with full-width matmuls corrupt nondeterministically on
    real TRN2 hardware (verified; CoreSim is clean).
  - DVE/ACT (alternating) evict each result bank; DMA to
    outT[bank*32 + row, b]; the host inverse-permutes pair rows and
    concatenates the batch shards.
  - POOL_BANKS can route whole banks' e-reduction to GPSIMD
    (tensor_reduce axis=C per 64-partition half-group, HW-verified
    correct); currently disabled — the drain engines, not the PE, set
    the wall in those stretches, so the offload bought no wall time.
"""

import numpy as np
import ml_dtypes

import concourse.bass as bass
import concourse.mybir as mybir
import concourse.tile as tile
from concourse import bacc
from concourse.bass_utils import run_bass_kernel_spmd

B, F, D = 2048, 40, 64
P = F * (F - 1) // 2  # 780
NCORES = 8
B_LOC = B // NCORES  # 256
F32 = mybir.dt.float32
BF16 = mybir.dt.bfloat16

NCHUNK = F // 2  # 20 xT chunks (2 fields each)
# tile list: (t, i) — pairs (i, 2t) [dummy if i==2t] and (i, 2t+1)
TILES = [(t, i) for t in range(NCHUNK) for i in range(2 * t + 1)]
NTILES = len(TILES)  # 400
REDUCE_COLTILE = False
TILES_PER_BANK = 64 if REDUCE_COLTILE else 16
BANK_ROWS = 128 if REDUCE_COLTILE else 32
NBANKS = (NTILES + TILES_PER_BANK - 1) // TILES_PER_BANK
OUT_ROWS = NBANKS * BANK_ROWS


GMAX = 4  # tiles per PSUM group (4 x 256 cols = 2 banks)
REDUCE_DELAY = 9  # groups of reduce-matmul lag (software pipelining)

# DMA prefetch slicing (units: xtlo = fields, xtc/xtcb = chunks). The first
# pieces are small so the first stage-1/mult tiles unblock early; x DMAs ride
# the ACT HWDGE queue so they overlap W DMAs issued on the SP queue.
XTLO_PIECES = [(0, 8), (8, 16), (16, 28), (28, 40)]
XCHUNK_PIECES = [(0, 4), (4, 8), (8, 14), (14, 20)]
WDMA_PREFETCH = 4  # W batches issued before the compute loop


def _build_groups():
    # pairs of same-chunk tiles sharing one [128,512] PSUM bank; split at
    # bank boundaries so both reduce slots land in the same bank-pass
    groups = []
    k = 0
    for t in range(NCHUNK):
        ilist = list(range(2 * t + 1))
        while ilist:
            take = min(GMAX, len(ilist), TILES_PER_BANK - (k % TILES_PER_BANK))
            groups.append((t, ilist[:take]))
            ilist = ilist[take:]
            k += take
    return groups


GROUPS = _build_groups()

# Banks whose e-reduction runs on GPSIMD (partition-direction tensor_reduce,
# axis=C, one call per 64-partition half-group) instead of PE ones-matmuls.
# Offloads ~16% of the PE's reduce pass onto the otherwise idle Pool engine.
POOL_BANKS = ()
PO_ROW = 1024  # porow cols per half-group slot (GMAX * B_LOC)


def _bank_groups():
    out = {}
    k = 0
    for t, ilist in GROUPS:
        gsz = len(ilist)
        out.setdefault(k // TILES_PER_BANK, []).append((k, gsz))
        k += gsz
    return out


BANK_GROUPS = _bank_groups()
NPOOLB = max(1, len(POOL_BANKS))  # >=1 so the dram tensor stays valid
PO_NG = max([len(BANK_GROUPS[b]) for b in POOL_BANKS], default=1)  # groups per pool bank
PO_COLS = 2 * PO_NG * PO_ROW

WDMA_BATCH = 16  # stage-1 lhsT tiles per DMA


def host_prep(W: np.ndarray):
    """Build Wt3 [64, NTILES*128] f32, ONES [128, 512] bf16, PERM info."""
    # Wt2[d, p, e]
    Wt2 = np.ascontiguousarray(W.transpose(1, 0, 2))  # [64, 780, 64]
    pair_idx = -np.ones((F, F), dtype=np.int64)
    k = 0
    for i in range(F):
        for j in range(i + 1, F):
            pair_idx[i, j] = k
            k += 1
    Wt3 = np.zeros((D, NTILES * 128), dtype=np.float32)  # cast to bf16 at end
    # rows[k] = (origA or -1, origB) for tile k
    rows = []
    for k, (t, i) in enumerate(TILES):
        jA, jB = 2 * t, 2 * t + 1
        pA = pair_idx[i, jA] if i < jA else -1
        pB = pair_idx[i, jB]
        if pA >= 0:
            Wt3[:, k * 128 : k * 128 + 64] = Wt2[:, pA, :]
        Wt3[:, k * 128 + 64 : k * 128 + 128] = Wt2[:, pB, :]
        rows.append((pA, pB))
    # ones masks: ONES[:, q*32+m] — slot q (0..15): col 2q active for k<64,
    # col 2q+1 active for k>=64
    ones = np.zeros((128, 512), dtype=np.float32)
    for q in range(16):
        ones[0:64, q * 32 + 2 * q] = 1.0
        ones[64:128, q * 32 + 2 * q + 1] = 1.0
    ones = ones.astype(ml_dtypes.bfloat16)
    # out row of tile k: bank = k//64, s = k%64, g = s%4, q = s//4
    # rowA = bank*128 + 32*g + 2*q ; rowB = rowA + 1
    perm_src = np.zeros(P, dtype=np.int64)  # outT row for original pair p
    # pool-bank pairs come from poolout[pb, (gi*2+h)*PO_ROW + idx*B_LOC]
    pool_src = {}  # orig pair p -> (pb, col)
    kg = {}
    for b, glist in BANK_GROUPS.items():
        for gi, (k0, gsz) in enumerate(glist):
            for idx in range(gsz):
                kg[k0 + idx] = (b, gi, idx)
    for k, (pA, pB) in enumerate(rows):
        bank, s = divmod(k, TILES_PER_BANK)
        if bank in POOL_BANKS:
            pb = POOL_BANKS.index(bank)
            _, gi, idx = kg[k]
            if pA >= 0:
                pool_src[pA] = (pb, gi * 2 * PO_ROW + idx * B_LOC)
            pool_src[pB] = (pb, (gi * 2 + 1) * PO_ROW + idx * B_LOC)
            continue
        if REDUCE_COLTILE:
            g, q = s % 4, s // 4
            rowA = bank * BANK_ROWS + 32 * g + 2 * q
        else:
            rowA = bank * BANK_ROWS + 2 * s
        if pA >= 0:
            perm_src[pA] = rowA
        perm_src[pB] = rowA + 1
    return Wt3.astype(ml_dtypes.bfloat16), ones, perm_src, pool_src


def build_nc():
    nc = bacc.Bacc("TRN2", target_bir_lowering=False, debug=False)

    xtlo_dram = nc.dram_tensor(
        "xtlo", [64, F * B_LOC], BF16, kind="ExternalInput"
    ).ap()
    xtcb_dram = nc.dram_tensor(
        "xtcb", [128, NCHUNK * B_LOC], BF16, kind="ExternalInput"
    ).ap()
    wt_dram = nc.dram_tensor("Wt3", [D, NTILES * 128], BF16, kind="ExternalInput").ap()
    ones_dram = nc.dram_tensor("ones", [128, 512], BF16, kind="ExternalInput").ap()
    out_dram = nc.dram_tensor("outT", [OUT_ROWS, B_LOC], F32, kind="ExternalOutput").ap()
    po_dram = nc.dram_tensor(
        "poolout", [NPOOLB, PO_COLS], F32, kind="ExternalOutput"
    ).ap()

    with tile.TileContext(nc) as tc:
        with (
            tc.tile_pool(name="persist", bufs=1) as persist,
            tc.tile_pool(name="wpool", bufs=4) as wpool,
            tc.tile_pool(name="zpool", bufs=17) as zpool,
            tc.tile_pool(name="ybfpool", bufs=6) as ybfpool,
            tc.tile_pool(name="opool", bufs=2) as opool,
            tc.tile_pool(name="popool", bufs=2) as popool,
            tc.tile_pool(name="ypsum", bufs=3, space=bass.MemorySpace.PSUM) as ypsum,
            tc.tile_pool(name="rpsum", bufs=2, space=bass.MemorySpace.PSUM) as rpsum,
        ):
            ones = persist.tile([128, 512], BF16, tag="ones")

            # XTC[(f%2)*64 + d, t*256 + m*128 + b]  (f = 2t + f%2) and the
            # low-half layout (all fields at partitions 0-63) are both
            # pre-transposed on the host and DMA'd directly.
            xtlo = persist.tile([64, F * B_LOC], BF16, tag="xtlo")
            xtcb = persist.tile([128, NCHUNK * B_LOC], BF16, tag="xtcb")

            # W batches are DMA'd on the SP queue; x layouts ride the ACT
            # HWDGE queue so both descriptor-gen streams overlap. Pieces are
            # issued smallest-first so early tiles unblock within ~2us.
            wtiles = []

            def w_dma(bi):
                kt0 = bi * WDMA_BATCH
                nw = min(WDMA_BATCH, NTILES - kt0)
                wt = wpool.tile([64, WDMA_BATCH * 128], BF16, tag="w")
                nc.sync.dma_start(
                    out=wt[:, : nw * 128],
                    in_=wt_dram[:, kt0 * 128 : (kt0 + nw) * 128],
                )
                wtiles.append(wt)

            def x_dma(piece, eng):
                kind, lo, hi = piece
                src, dst = {
                    "xtlo": (xtlo_dram, xtlo),
                    "xtcb": (xtcb_dram, xtcb),
                }[kind]
                eng.dma_start(
                    out=dst[:, lo * B_LOC : hi * B_LOC],
                    in_=src[:, lo * B_LOC : hi * B_LOC],
                )

            # First pieces on the ACT HWDGE queue (3 gens before the first
            # evict dispatch); the tail rides Pool SWDGE, which is idle until
            # the out-DMAs begin.
            w_dma(0)
            x_dma(("xtlo",) + XTLO_PIECES[0], nc.scalar)
            x_dma(("xtcb",) + XCHUNK_PIECES[0], nc.scalar)
            w_dma(1)
            x_dma(("xtcb",) + XCHUNK_PIECES[1], nc.gpsimd)
            x_dma(("xtlo",) + XTLO_PIECES[1], nc.scalar)
            w_dma(2)
            x_dma(("xtcb",) + XCHUNK_PIECES[2], nc.gpsimd)
            x_dma(("xtlo",) + XTLO_PIECES[2], nc.gpsimd)
            w_dma(3)
            x_dma(("xtcb",) + XCHUNK_PIECES[3], nc.gpsimd)
            x_dma(("xtlo",) + XTLO_PIECES[3], nc.gpsimd)
            nc.sync.dma_start(out=ones[:], in_=ones_dram[:])

            rbs = [None]
            k = 0
            nquad = 0
            pending = []

            porows = [None]

            def emit_reduce(z, k0, gsz):
                bank0 = k0 // TILES_PER_BANK
                if bank0 in POOL_BANKS:
                    # GPSIMD partition reduce: one [64, gsz*256] half-group
                    # per call -> [1, gsz*256] pair-rows on partition 0
                    glist = BANK_GROUPS[bank0]
                    gi = [g[0] for g in glist].index(k0)
                    if gi == 0:
                        porows[0] = popool.tile([1, PO_COLS], F32, tag="po", name="po")
                    po = porows[0]
                    for h in range(2):
                        c0 = (gi * 2 + h) * PO_ROW
                        nc.gpsimd.tensor_reduce(
                            out=po[:, c0 : c0 + gsz * B_LOC],
                            in_=z[h * 64 : (h + 1) * 64, : gsz * B_LOC],
                            axis=mybir.AxisListType.C,
                            op=mybir.AluOpType.add,
                        )
                    if gi == len(glist) - 1:
                        pb = POOL_BANKS.index(bank0)
                        nc.sync.dma_start(
                            out=po_dram[pb : pb + 1, :], in_=po[:]
                        )
                    return
                # accumulate into reduce bank via ones-mask matmuls
                for idx in range(gsz):
                    kt = k0 + idx
                    bank, s = divmod(kt, TILES_PER_BANK)
                    q = s
                    if s == 0:
                        rbs[0] = rpsum.tile([128, B_LOC], F32, tag="rb", name="rb")
                    rb = rbs[0]
                    last_in_bank = (s == TILES_PER_BANK - 1) or (kt == NTILES - 1)
                    nc.tensor.matmul(
                        rb[0:32, :],
                        ones[:, q * 32 : (q + 1) * 32],
                        z[:, idx * B_LOC : (idx + 1) * B_LOC],
                        start=(s == 0),
                        stop=last_in_bank,
                        tile_position=(0, 0),
                        skip_group_check=True,
                    )
                    if last_in_bank:
                        ob = opool.tile([BANK_ROWS, B_LOC], F32, tag="ob")
                        # alternate the result-bank eviction between DVE and
                        # ACT so neither exceeds its steady-state budget
                        if bank % 2 == 0 and bank < NBANKS - 2:
                            nc.vector.tensor_copy(out=ob[:], in_=rb[0:BANK_ROWS, :])
                        else:
                            nc.scalar.copy(out=ob[:], in_=rb[0:BANK_ROWS, :])
                        # SWDGE (Pool) out-DMA: keeps the SP queue free for W
                        # batches and dodges the shared HWDGE generator. The
                        # last banks ride SP (idle by then, cheaper gen) to
                        # shorten the drain tail.
                        oeng = nc.sync if bank >= NBANKS - 2 else nc.gpsimd
                        oeng.dma_start(
                            out=out_dram[
                                bank * BANK_ROWS : (bank + 1) * BANK_ROWS, :
                            ],
                            in_=ob[:],
                        )

            for t, ilist in GROUPS:
                gsz = len(ilist)
                # stage 1: Y[(p, e), b] = Wtile.T @ xT_i — gsz tiles share
                # one PSUM bank (disjoint column halves)
                y = ypsum.tile([128, GMAX * B_LOC], F32, tag="y")
                for idx, i in enumerate(ilist):
                    kt = k + idx
                    if kt % WDMA_BATCH == 0:
                        bi = kt // WDMA_BATCH + 2  # two batches of DMA lead
                        if WDMA_PREFETCH <= bi < (NTILES + WDMA_BATCH - 1) // WDMA_BATCH:
                            w_dma(bi)
                    kk = kt % WDMA_BATCH
                    wchunk = wtiles[kt // WDMA_BATCH]
                    nc.tensor.matmul(
                        y[:, idx * B_LOC : (idx + 1) * B_LOC],
                        wchunk[:, kk * 128 : (kk + 1) * 128],
                        xtlo[:, i * B_LOC : (i + 1) * B_LOC],
                        start=True,
                        stop=True,
                    )

                # stage 2: z = Y * xT[j-fields chunk t]  (bf16 out), one TT
                # per group with stride-0 broadcast of the xtc chunk.
                # A fraction of pair-groups takes the ACT-evict + GPSIMD
                # multiply path to unload the DVE.
                z = zpool.tile([128, GMAX * B_LOC], BF16, tag="z")
                if gsz > 1:
                    nquad += 1
                    act_path = gsz == GMAX and nquad % 4 != 1
                    if act_path:
                        # ACT evicts PSUM -> bf16, DVE multiplies at 2x
                        ybf = ybfpool.tile([128, GMAX * B_LOC], BF16, tag="ybf")
                        nc.scalar.copy(out=ybf[:], in_=y[:])
                        in1 = xtcb[
                            :, None, t * B_LOC : (t + 1) * B_LOC
                        ].to_broadcast([128, gsz, B_LOC])
                        nc.vector.tensor_tensor(
                            z[:, : gsz * B_LOC].rearrange(
                                "p (n b) -> p n b", n=gsz
                            ),
                            ybf[:, : gsz * B_LOC].rearrange(
                                "p (n b) -> p n b", n=gsz
                            ),
                            in1,
                            mybir.AluOpType.mult,
                        )
                    else:
                        # every 9th quad drains fused on the DVE (PSUM f32
                        # multiply at 1x) to keep ACT under the PE roofline
                        in1 = xtcb[
                            :, None, t * B_LOC : (t + 1) * B_LOC
                        ].to_broadcast([128, gsz, B_LOC])
                        nc.vector.tensor_tensor(
                            z[:, : gsz * B_LOC].rearrange(
                                "p (n b) -> p n b", n=gsz
                            ),
                            y[:, : gsz * B_LOC].rearrange(
                                "p (n b) -> p n b", n=gsz
                            ),
                            in1,
                            mybir.AluOpType.mult,
                        )
                else:
                    nc.vector.tensor_tensor(
                        z[:, :B_LOC],
                        y[:, :B_LOC],
                        xtcb[:, t * B_LOC : (t + 1) * B_LOC],
                        mybir.AluOpType.mult,
                    )

                # stage 3 is software-pipelined: queue this group's reduce
                # and emit the one from REDUCE_DELAY groups ago, so the PE's
                # in-order queue never waits on the just-issued ACT->DVE
                # multiply chain.
                pending.append((z, k, gsz))
                if len(pending) > REDUCE_DELAY:
                    emit_reduce(*pending.pop(0))
                k += gsz

            while pending:
                emit_reduce(*pending.pop(0))

    nc.compile()
    return nc


_NC = None


def kernel(x: np.ndarray, W: np.ndarray) -> np.ndarray:
    global _NC
    x = np.ascontiguousarray(np.asarray(x, dtype=np.float32))
    W = np.ascontiguousarray(np.asarray(W, dtype=np.float32))
    assert x.shape == (B, F, D) and W.shape == (P, D, D)

    Wt3, ones, perm_src, pool_src = host_prep(W)

    if _NC is None:
        _NC = build_nc()

    in_maps = []
    for c in range(NCORES):
        xs = x[c * B_LOC : (c + 1) * B_LOC]  # [256, 40, 64]
        v = xs.transpose(1, 2, 0).reshape(NCHUNK, 2, D, B_LOC)
        xtc = np.ascontiguousarray(
            v.transpose(1, 2, 0, 3).reshape(128, NCHUNK * B_LOC)
        )
        xtlo = np.ascontiguousarray(
            xs.transpose(2, 1, 0).reshape(D, F * B_LOC)
        ).astype(ml_dtypes.bfloat16)
        xtcb = xtc.astype(ml_dtypes.bfloat16)
        in_maps.append(
            {"xtcb": xtcb, "xtlo": xtlo, "Wt3": Wt3, "ones": ones}
        )
    res = run_bass_kernel_spmd(_NC, in_maps, core_ids=list(range(NCORES)))
    out = np.empty((B, P), dtype=np.float32)
    for c in range(NCORES):
        outT = res.results[c]["outT"]  # [OUT_ROWS, B_LOC]
        out[c * B_LOC : (c + 1) * B_LOC, :] = outT[perm_src, :].T
        po = res.results[c]["poolout"]  # [NPOOLB, PO_COLS]
        for p, (pb, col) in pool_src.items():
            out[c * B_LOC : (c + 1) * B_LOC, p] = po[pb, col : col + B_LOC]
    return out



# revision 60
# speedup vs baseline: 1.2306x; 1.0863x over previous
"""Trainium2 Bass kernel for BilinearInteraction.

out[b, p] = x[b, i_p, :] @ W[p] @ x[b, j_p, :]  for the 780 field pairs
(i, j), i < j, of F=40 fields (row-major triu order).

Architecture (8 NeuronCores, data-parallel over batch, B_loc=256):
  - "b-T" layout: stage-1 PE matmuls produce Y[(pair, e), b] in PSUM
    (pairs x e on partitions, batch on the free dim), so the final
    e-reduction runs on the PE as ones-mask matmuls (contraction over
    partitions), keeping the vector engine to a single multiply pass.
  - Tiles: one [128, 256] PSUM slice holds 2 pairs sharing one i-field:
    (i, 2t) and (i, 2t+1), matching xT chunk t (fields 2t / 2t+1 on the
    two partition halves). W is host-permuted (bf16) into per-tile
    contiguous lhsT blocks (zero blocks for invalid (i==2t, 2t) slots).
    4 tiles of one chunk share a 2-bank [128, 1024] PSUM group so one
    tensor_tensor covers 4 tiles (amortizes the DVE PSUM-access bubble).
  - Host pre-transposes x into the two layouts the kernel needs
    (xtcb bf16 for the multiply, xtlo bf16 for stage-1 rhs),
    eliminating all on-device transposes.
  - DMA queues: W batches + ones on SP, x pieces split across the ACT
    HWDGE queue (first pieces) and Pool SWDGE (tail), out-DMAs on Pool
    SWDGE (last two on SP to shorten the drain tail). Pieces are
    ordered so the first stage-1/mult tiles unblock within ~3.5us.
  - stage 1: PE matmul Y = Wtile.T @ xT_i  (bf16, K=64, M=128, N=256).
  - stage 2: ACT evicts Y -> bf16 SBUF; DVE multiplies by xtcb chunk at
    the 2x packed rate -> z bf16. Every 4th quad (and the small
    leftover groups) instead drains fused on the DVE (PSUM f32 x bf16
    at 1x) because a pure ACT evict path (1038ns/quad) cannot keep up
    with the PE's 854ns/quad group cadence.
  - stage 3: PE ones-mask matmuls (K=128, M=32) accumulate 16 z-tiles
    into one PSUM bank = 32 output pair-rows (full fp32 accumulation).
    Col-group tiling is deliberately NOT used: tile_position col-groups
    interleaved with full-width matmuls corrupt nondeterministically on
    real TRN2 hardware (verified; CoreSim is clean).
  - DVE/ACT (alternating) evict each result bank; DMA to
    outT[bank*32 + row, b]; the host inverse-permutes pair rows and
    concatenates the batch shards.
  - POOL_BANKS can route whole banks' e-reduction to GPSIMD
    (tensor_reduce axis=C per 64-partition half-group, HW-verified
    correct); currently disabled — the drain engines, not the PE, set
    the wall in those stretches, so the offload bought no wall time.
"""

import numpy as np
import ml_dtypes

import concourse.bass as bass
import concourse.mybir as mybir
import concourse.tile as tile
from concourse import bacc
from concourse.bass_utils import run_bass_kernel_spmd

B, F, D = 2048, 40, 64
P = F * (F - 1) // 2  # 780
NCORES = 8
B_LOC = B // NCORES  # 256
F32 = mybir.dt.float32
BF16 = mybir.dt.bfloat16

NCHUNK = F // 2  # 20 xT chunks (2 fields each)
# tile list: (t, i) — pairs (i, 2t) [dummy if i==2t] and (i, 2t+1)
TILES = [(t, i) for t in range(NCHUNK) for i in range(2 * t + 1)]
NTILES = len(TILES)  # 400
REDUCE_COLTILE = False
TILES_PER_BANK = 64 if REDUCE_COLTILE else 16
BANK_ROWS = 128 if REDUCE_COLTILE else 32
NBANKS = (NTILES + TILES_PER_BANK - 1) // TILES_PER_BANK
OUT_ROWS = NBANKS * BANK_ROWS


GMAX = 4  # tiles per PSUM group (4 x 256 cols = 2 banks)
REDUCE_DELAY = 12  # groups of reduce-matmul lag (software pipelining)

# DMA prefetch slicing (units: xtlo = fields, xtc/xtcb = chunks). The first
# pieces are small so the first stage-1/mult tiles unblock early; x DMAs ride
# the ACT HWDGE queue so they overlap W DMAs issued on the SP queue.
XTLO_PIECES = [(0, 2), (2, 8), (8, 16), (16, 28), (28, 40)]
XCHUNK_PIECES = [(0, 4), (4, 8), (8, 14), (14, 20)]
WDMA_PREFETCH = 4  # W batches issued before the compute loop


def _build_groups():
    # pairs of same-chunk tiles sharing one [128,512] PSUM bank; split at
    # bank boundaries so both reduce slots land in the same bank-pass
    groups = []
    k = 0
    for t in range(NCHUNK):
        ilist = list(range(2 * t + 1))
        while ilist:
            take = min(GMAX, len(ilist), TILES_PER_BANK - (k % TILES_PER_BANK))
            groups.append((t, ilist[:take]))
            ilist = ilist[take:]
            k += take
    return groups


GROUPS = _build_groups()

# Banks whose e-reduction runs on GPSIMD (partition-direction tensor_reduce,
# axis=C, one call per 64-partition half-group) instead of PE ones-matmuls.
# Offloads ~16% of the PE's reduce pass onto the otherwise idle Pool engine.
POOL_BANKS = ()
PO_ROW = 1024  # porow cols per half-group slot (GMAX * B_LOC)


def _bank_groups():
    out = {}
    k = 0
    for t, ilist in GROUPS:
        gsz = len(ilist)
        out.setdefault(k // TILES_PER_BANK, []).append((k, gsz))
        k += gsz
    return out


BANK_GROUPS = _bank_groups()
NPOOLB = max(1, len(POOL_BANKS))  # >=1 so the dram tensor stays valid
PO_NG = max([len(BANK_GROUPS[b]) for b in POOL_BANKS], default=1)  # groups per pool bank
PO_COLS = 2 * PO_NG * PO_ROW

WDMA_BATCH = 16  # stage-1 lhsT tiles per DMA


def host_prep(W: np.ndarray):
    """Build Wt3 [64, NTILES*128] f32, ONES [128, 512] bf16, PERM info."""
    # Wt2[d, p, e]
    Wt2 = np.ascontiguousarray(W.transpose(1, 0, 2))  # [64, 780, 64]
    pair_idx = -np.ones((F, F), dtype=np.int64)
    k = 0
    for i in range(F):
        for j in range(i + 1, F):
            pair_idx[i, j] = k
            k += 1
    Wt3 = np.zeros((D, NTILES * 128), dtype=np.float32)  # cast to bf16 at end
    # rows[k] = (origA or -1, origB) for tile k
    rows = []
    for k, (t, i) in enumerate(TILES):
        jA, jB = 2 * t, 2 * t + 1
        pA = pair_idx[i, jA] if i < jA else -1
        pB = pair_idx[i, jB]
        if pA >= 0:
            Wt3[:, k * 128 : k * 128 + 64] = Wt2[:, pA, :]
        Wt3[:, k * 128 + 64 : k * 128 + 128] = Wt2[:, pB, :]
        rows.append((pA, pB))
    # ones masks: ONES[:, q*32+m] — slot q (0..15): col 2q active for k<64,
    # col 2q+1 active for k>=64
    ones = np.zeros((128, 512), dtype=np.float32)
    for q in range(16):
        ones[0:64, q * 32 + 2 * q] = 1.0
        ones[64:128, q * 32 + 2 * q + 1] = 1.0
    ones = ones.astype(ml_dtypes.bfloat16)
    # out row of tile k: bank = k//64, s = k%64, g = s%4, q = s//4
    # rowA = bank*128 + 32*g + 2*q ; rowB = rowA + 1
    perm_src = np.zeros(P, dtype=np.int64)  # outT row for original pair p
    # pool-bank pairs come from poolout[pb, (gi*2+h)*PO_ROW + idx*B_LOC]
    pool_src = {}  # orig pair p -> (pb, col)
    kg = {}
    for b, glist in BANK_GROUPS.items():
        for gi, (k0, gsz) in enumerate(glist):
            for idx in range(gsz):
                kg[k0 + idx] = (b, gi, idx)
    for k, (pA, pB) in enumerate(rows):
        bank, s = divmod(k, TILES_PER_BANK)
        if bank in POOL_BANKS:
            pb = POOL_BANKS.index(bank)
            _, gi, idx = kg[k]
            if pA >= 0:
                pool_src[pA] = (pb, gi * 2 * PO_ROW + idx * B_LOC)
            pool_src[pB] = (pb, (gi * 2 + 1) * PO_ROW + idx * B_LOC)
            continue
        if REDUCE_COLTILE:
            g, q = s % 4, s // 4
            rowA = bank * BANK_ROWS + 32 * g + 2 * q
        else:
            rowA = bank * BANK_ROWS + 2 * s
        if pA >= 0:
            perm_src[pA] = rowA
        perm_src[pB] = rowA + 1
    return Wt3.astype(ml_dtypes.bfloat16), ones, perm_src, pool_src


def build_nc():
    nc = bacc.Bacc("TRN2", target_bir_lowering=False, debug=False)

    xtlo_dram = nc.dram_tensor(
        "xtlo", [64, F * B_LOC], BF16, kind="ExternalInput"
    ).ap()
    xtcb_dram = nc.dram_tensor(
        "xtcb", [128, NCHUNK * B_LOC], BF16, kind="ExternalInput"
    ).ap()
    wt_dram = nc.dram_tensor("Wt3", [D, NTILES * 128], BF16, kind="ExternalInput").ap()
    ones_dram = nc.dram_tensor("ones", [128, 512], BF16, kind="ExternalInput").ap()
    out_dram = nc.dram_tensor("outT", [OUT_ROWS, B_LOC], F32, kind="ExternalOutput").ap()
    po_dram = nc.dram_tensor(
        "poolout", [NPOOLB, PO_COLS], F32, kind="ExternalOutput"
    ).ap()

    with tile.TileContext(nc) as tc:
        with (
            tc.tile_pool(name="persist", bufs=1) as persist,
            tc.tile_pool(name="wpool", bufs=4) as wpool,
            tc.tile_pool(name="zpool", bufs=20) as zpool,
            tc.tile_pool(name="ybfpool", bufs=6) as ybfpool,
            tc.tile_pool(name="opool", bufs=2) as opool,
            tc.tile_pool(name="popool", bufs=2) as popool,
            tc.tile_pool(name="ypsum", bufs=3, space=bass.MemorySpace.PSUM) as ypsum,
            tc.tile_pool(name="rpsum", bufs=1, space=bass.MemorySpace.PSUM) as rpsum,
        ):
            ones = persist.tile([128, 512], BF16, tag="ones")

            # XTC[(f%2)*64 + d, t*256 + m*128 + b]  (f = 2t + f%2) and the
            # low-half layout (all fields at partitions 0-63) are both
            # pre-transposed on the host and DMA'd directly.
            xtlo = persist.tile([64, F * B_LOC], BF16, tag="xtlo")
            xtcb = persist.tile([128, NCHUNK * B_LOC], BF16, tag="xtcb")

            # W batches are DMA'd on the SP queue; x layouts ride the ACT
            # HWDGE queue so both descriptor-gen streams overlap. Pieces are
            # issued smallest-first so early tiles unblock within ~2us.
            wtiles = []

            def w_dma(bi):
                kt0 = bi * WDMA_BATCH
                nw = min(WDMA_BATCH, NTILES - kt0)
                wt = wpool.tile([64, WDMA_BATCH * 128], BF16, tag="w")
                if bi == 0:
                    # two half-DMAs: tiles 0-7 land ~0.4us sooner
                    nc.sync.dma_start(out=wt[:, : 8 * 128], in_=wt_dram[:, : 8 * 128])
                    nc.sync.dma_start(
                        out=wt[:, 8 * 128 : nw * 128],
                        in_=wt_dram[:, 8 * 128 : nw * 128],
                    )
                else:
                    nc.sync.dma_start(
                        out=wt[:, : nw * 128],
                        in_=wt_dram[:, kt0 * 128 : (kt0 + nw) * 128],
                    )
                wtiles.append(wt)

            def x_dma(piece, eng):
                kind, lo, hi = piece
                src, dst = {
                    "xtlo": (xtlo_dram, xtlo),
                    "xtcb": (xtcb_dram, xtcb),
                }[kind]
                eng.dma_start(
                    out=dst[:, lo * B_LOC : hi * B_LOC],
                    in_=src[:, lo * B_LOC : hi * B_LOC],
                )

            # First pieces on the ACT HWDGE queue (3 gens before the first
            # evict dispatch); the tail rides Pool SWDGE, which is idle until
            # the out-DMAs begin.
            w_dma(0)
            x_dma(("xtlo",) + XTLO_PIECES[0], nc.scalar)
            nc.gpsimd.dma_start(out=ones[:], in_=ones_dram[:])
            x_dma(("xtcb",) + XCHUNK_PIECES[0], nc.gpsimd)
            x_dma(("xtlo",) + XTLO_PIECES[1], nc.scalar)
            w_dma(1)
            x_dma(("xtcb",) + XCHUNK_PIECES[1], nc.gpsimd)
            x_dma(("xtlo",) + XTLO_PIECES[2], nc.scalar)
            w_dma(2)
            x_dma(("xtcb",) + XCHUNK_PIECES[2], nc.gpsimd)
            x_dma(("xtlo",) + XTLO_PIECES[3], nc.gpsimd)
            w_dma(3)
            x_dma(("xtcb",) + XCHUNK_PIECES[3], nc.gpsimd)
            x_dma(("xtlo",) + XTLO_PIECES[4], nc.gpsimd)

            # one persistent 2-bank reduce tile, 4 slots of [0:32, 256]:
            # bank b -> slot ((b>>1)&1)*2 + (b&1); pairs of banks evict in a
            # single [32, 512] ACT copy + one DMA (frees the DVE entirely)
            rball = rpsum.tile([128, 4 * B_LOC], F32, tag="rball", name="rball")
            k = 0
            nquad = 0
            pending = []

            porows = [None]

            def emit_reduce(z, k0, gsz):
                bank0 = k0 // TILES_PER_BANK
                if bank0 in POOL_BANKS:
                    # GPSIMD partition reduce: one [64, gsz*256] half-group
                    # per call -> [1, gsz*256] pair-rows on partition 0
                    glist = BANK_GROUPS[bank0]
                    gi = [g[0] for g in glist].index(k0)
                    if gi == 0:
                        porows[0] = popool.tile([1, PO_COLS], F32, tag="po", name="po")
                    po = porows[0]
                    for h in range(2):
                        c0 = (gi * 2 + h) * PO_ROW
                        nc.gpsimd.tensor_reduce(
                            out=po[:, c0 : c0 + gsz * B_LOC],
                            in_=z[h * 64 : (h + 1) * 64, : gsz * B_LOC],
                            axis=mybir.AxisListType.C,
                            op=mybir.AluOpType.add,
                        )
                    if gi == len(glist) - 1:
                        pb = POOL_BANKS.index(bank0)
                        nc.sync.dma_start(
                            out=po_dram[pb : pb + 1, :], in_=po[:]
                        )
                    return
                # accumulate into reduce bank via ones-mask matmuls
                for idx in range(gsz):
                    kt = k0 + idx
                    bank, s = divmod(kt, TILES_PER_BANK)
                    q = s
                    slot = ((bank >> 1) & 1) * 2 + (bank & 1)
                    rb = rball[:, slot * B_LOC : (slot + 1) * B_LOC]
                    last_in_bank = (s == TILES_PER_BANK - 1) or (kt == NTILES - 1)
                    nc.tensor.matmul(
                        rb[0:32, :],
                        ones[:, q * 32 : (q + 1) * 32],
                        z[:, idx * B_LOC : (idx + 1) * B_LOC],
                        start=(s == 0),
                        stop=last_in_bank,
                        tile_position=(0, 0),
                        skip_group_check=True,
                    )
                    if last_in_bank and bank % 2 == 1:
                        ps = (slot // 2) * 2
                        ob = opool.tile([BANK_ROWS, 2 * B_LOC], F32, tag="ob")
                        nc.scalar.copy(
                            out=ob[:],
                            in_=rball[0:BANK_ROWS, ps * B_LOC : (ps + 2) * B_LOC],
                        )
                        # SWDGE (Pool) out-DMA keeps the SP queue free for W
                        # batches; the last pair rides SP (idle by then) to
                        # shorten the drain tail.
                        oeng = nc.sync if bank >= NBANKS - 3 else nc.gpsimd
                        oeng.dma_start(
                            out=out_dram[
                                (bank - 1) * BANK_ROWS : (bank + 1) * BANK_ROWS, :
                            ].rearrange("(s p) c -> p s c", s=2),
                            in_=ob[:].rearrange("p (s c) -> p s c", s=2),
                        )
                    elif last_in_bank and kt == NTILES - 1:
                        ob = opool.tile([BANK_ROWS, B_LOC], F32, tag="ob")
                        nc.scalar.copy(out=ob[:], in_=rb[0:BANK_ROWS, :])
                        nc.sync.dma_start(
                            out=out_dram[
                                bank * BANK_ROWS : (bank + 1) * BANK_ROWS, :
                            ],
                            in_=ob[:],
                        )

            for gidx, (t, ilist) in enumerate(GROUPS):
                gsz = len(ilist)
                # stage 1: Y[(p, e), b] = Wtile.T @ xT_i — gsz tiles share
                # one PSUM bank (disjoint column halves)
                y = ypsum.tile([128, GMAX * B_LOC], F32, tag="y")
                for idx, i in enumerate(ilist):
                    kt = k + idx
                    if kt % WDMA_BATCH == 0:
                        bi = kt // WDMA_BATCH + 2  # two batches of DMA lead
                        if WDMA_PREFETCH <= bi < (NTILES + WDMA_BATCH - 1) // WDMA_BATCH:
                            w_dma(bi)
                    kk = kt % WDMA_BATCH
                    wchunk = wtiles[kt // WDMA_BATCH]
                    nc.tensor.matmul(
                        y[:, idx * B_LOC : (idx + 1) * B_LOC],
                        wchunk[:, kk * 128 : (kk + 1) * 128],
                        xtlo[:, i * B_LOC : (i + 1) * B_LOC],
                        start=True,
                        stop=True,
                    )

                # stage 2: z = Y * xT[j-fields chunk t]  (bf16 out), one TT
                # per group with stride-0 broadcast of the xtc chunk.
                # A fraction of pair-groups takes the ACT-evict + GPSIMD
                # multiply path to unload the DVE.
                z = zpool.tile([128, GMAX * B_LOC], BF16, tag="z")
                if True:
                    if gsz > 1:
                        nquad += 1
                    act_path = gsz == 1 or nquad % 4 != 1
                    if act_path:
                        # ACT evicts PSUM -> bf16, DVE multiplies at 2x
                        ybf = ybfpool.tile([128, GMAX * B_LOC], BF16, tag="ybf")
                        nc.scalar.copy(
                            out=ybf[:, : gsz * B_LOC], in_=y[:, : gsz * B_LOC]
                        )
                        in1 = xtcb[
                            :, None, t * B_LOC : (t + 1) * B_LOC
                        ].to_broadcast([128, gsz, B_LOC])
                        nc.vector.tensor_tensor(
                            z[:, : gsz * B_LOC].rearrange(
                                "p (n b) -> p n b", n=gsz
                            ),
                            ybf[:, : gsz * B_LOC].rearrange(
                                "p (n b) -> p n b", n=gsz
                            ),
                            in1,
                            mybir.AluOpType.mult,
                        )
                    else:
                        # every 9th quad drains fused on the DVE (PSUM f32
                        # multiply at 1x) to keep ACT under the PE roofline
                        in1 = xtcb[
                            :, None, t * B_LOC : (t + 1) * B_LOC
                        ].to_broadcast([128, gsz, B_LOC])
                        nc.vector.tensor_tensor(
                            z[:, : gsz * B_LOC].rearrange(
                                "p (n b) -> p n b", n=gsz
                            ),
                            y[:, : gsz * B_LOC].rearrange(
                                "p (n b) -> p n b", n=gsz
                            ),
                            in1,
                            mybir.AluOpType.mult,
                        )
                else:
                    nc.vector.tensor_tensor(
                        z[:, :B_LOC],
                        y[:, :B_LOC],
                        xtcb[:, t * B_LOC : (t + 1) * B_LOC],
                        mybir.AluOpType.mult,
                    )

                # stage 3 is software-pipelined: queue this group's reduce
                # and emit the one from REDUCE_DELAY groups ago, so the PE's
                # in-order queue never waits on the just-issued ACT->DVE
                # multiply chain.
                pending.append((z, k, gsz))
                if len(pending) > REDUCE_DELAY:
                    emit_reduce(*pending.pop(0))
                # taper the reduce lag over the final stretch so the last
                # banks' reduces interleave with stage-1 instead of running
                # as a pure PE flush after it
                if gidx >= len(GROUPS) - 3 * REDUCE_DELAY and len(pending) > 1:
                    emit_reduce(*pending.pop(0))
                k += gsz

            while pending:
                emit_reduce(*pending.pop(0))

    nc.compile()
    return nc


_NC = None


def kernel(x: np.ndarray, W: np.ndarray) -> np.ndarray:
    global _NC
    x = np.ascontiguousarray(np.asarray(x, dtype=np.float32))
    W = np.ascontiguousarray(np.asarray(W, dtype=np.float32))
    assert x.shape == (B, F, D) and W.shape == (P, D, D)

    Wt3, ones, perm_src, pool_src = host_prep(W)

    if _NC is None:
        _NC = build_nc()

    in_maps = []
    for c in range(NCORES):
        xs = x[c * B_LOC : (c + 1) * B_LOC]  # [256, 40, 64]
        v = xs.transpose(1, 2, 0).reshape(NCHUNK, 2, D, B_LOC)
        xtc = np.ascontiguousarray(
            v.transpose(1, 2, 0, 3).reshape(128, NCHUNK * B_LOC)
        )
        xtlo = np.ascontiguousarray(
            xs.transpose(2, 1, 0).reshape(D, F * B_LOC)
        ).astype(ml_dtypes.bfloat16)
        xtcb = xtc.astype(ml_dtypes.bfloat16)
        in_maps.append(
            {"xtcb": xtcb, "xtlo": xtlo, "Wt3": Wt3, "ones": ones}
        )
    res = run_bass_kernel_spmd(_NC, in_maps, core_ids=list(range(NCORES)))
    out = np.empty((B, P), dtype=np.float32)
    for c in range(NCORES):
        outT = res.results[c]["outT"]  # [OUT_ROWS, B_LOC]
        out[c * B_LOC : (c + 1) * B_LOC, :] = outT[perm_src, :].T
        po = res.results[c]["poolout"]  # [NPOOLB, PO_COLS]
        for p, (pb, col) in pool_src.items():
            out[c * B_LOC : (c + 1) * B_LOC, p] = po[pb, col : col + B_LOC]
    return out



# revision 67
# speedup vs baseline: 1.2322x; 1.0013x over previous
"""Trainium2 Bass kernel for BilinearInteraction.

out[b, p] = x[b, i_p, :] @ W[p] @ x[b, j_p, :]  for the 780 field pairs
(i, j), i < j, of F=40 fields (row-major triu order).

Architecture (8 NeuronCores, data-parallel over batch, B_loc=256):
  - "b-T" layout: stage-1 PE matmuls produce Y[(pair, e), b] in PSUM
    (pairs x e on partitions, batch on the free dim), so the final
    e-reduction runs on the PE as ones-mask matmuls (contraction over
    partitions), keeping the vector engine to a single multiply pass.
  - Tiles: one [128, 256] PSUM slice holds 2 pairs sharing one i-field:
    (i, 2t) and (i, 2t+1), matching xT chunk t (fields 2t / 2t+1 on the
    two partition halves). W is host-permuted (bf16) into per-tile
    contiguous lhsT blocks (zero blocks for invalid (i==2t, 2t) slots).
    4 tiles of one chunk share a 2-bank [128, 1024] PSUM group so one
    tensor_tensor covers 4 tiles (amortizes the DVE PSUM-access bubble).
  - Host pre-transposes x into the two layouts the kernel needs
    (xtcb bf16 for the multiply, xtlo bf16 for stage-1 rhs),
    eliminating all on-device transposes.
  - DMA queues: W batches + ones on SP, x pieces split across the ACT
    HWDGE queue (first pieces) and Pool SWDGE (tail), out-DMAs on Pool
    SWDGE (last two on SP to shorten the drain tail). Pieces are
    ordered so the first stage-1/mult tiles unblock within ~3.5us.
  - stage 1: PE matmul Y = Wtile.T @ xT_i  (bf16, K=64, M=128, N=256).
  - stage 2: ACT evicts Y -> bf16 SBUF (trimmed to the group's actual
    width); DVE multiplies by xtcb chunk at the 2x packed rate -> z
    bf16. All group sizes take this path except every 4th quad, which
    drains fused on the DVE (PSUM f32 x bf16 at 1x) because a pure ACT
    evict path (1038ns/quad) cannot keep up with the PE's 854ns/quad
    group cadence. The %4 spill is phase-locked to the 4-group banks.
  - stage 3: PE ones-mask matmuls (K=128, M=32) accumulate 16 z-tiles
    into one PSUM bank = 32 output pair-rows (full fp32 accumulation).
    Col-group tiling is deliberately NOT used: tile_position col-groups
    interleaved with full-width matmuls corrupt nondeterministically on
    real TRN2 hardware (verified; CoreSim is clean).
  - DVE/ACT (alternating) evict each result bank; DMA to
    outT[bank*32 + row, b]; the host inverse-permutes pair rows and
    concatenates the batch shards.
  - POOL_BANKS can route whole banks' e-reduction to GPSIMD
    (tensor_reduce axis=C per 64-partition half-group, HW-verified
    correct); currently disabled — the drain engines, not the PE, set
    the wall in those stretches, so the offload bought no wall time.
"""

import numpy as np
import ml_dtypes

import concourse.bass as bass
import concourse.mybir as mybir
import concourse.tile as tile
from concourse import bacc
from concourse.bass_utils import run_bass_kernel_spmd

B, F, D = 2048, 40, 64
P = F * (F - 1) // 2  # 780
NCORES = 8
B_LOC = B // NCORES  # 256
F32 = mybir.dt.float32
BF16 = mybir.dt.bfloat16

NCHUNK = F // 2  # 20 xT chunks (2 fields each)
# tile list: (t, i) — pairs (i, 2t) [dummy if i==2t] and (i, 2t+1)
TILES = [(t, i) for t in range(NCHUNK) for i in range(2 * t + 1)]
NTILES = len(TILES)  # 400
REDUCE_COLTILE = False
TILES_PER_BANK = 64 if REDUCE_COLTILE else 16
BANK_ROWS = 128 if REDUCE_COLTILE else 32
NBANKS = (NTILES + TILES_PER_BANK - 1) // TILES_PER_BANK
OUT_ROWS = NBANKS * BANK_ROWS


GMAX = 4  # tiles per PSUM group (4 x 256 cols = 2 banks)
REDUCE_DELAY = 12  # groups of reduce-matmul lag (software pipelining)

# DMA prefetch slicing (units: xtlo = fields, xtc/xtcb = chunks). The first
# pieces are small so the first stage-1/mult tiles unblock early; x DMAs ride
# the ACT HWDGE queue so they overlap W DMAs issued on the SP queue.
XTLO_PIECES = [(0, 2), (2, 8), (8, 16), (16, 28), (28, 40)]
XCHUNK_PIECES = [(0, 4), (4, 8), (8, 14), (14, 20)]
WDMA_PREFETCH = 4  # W batches issued before the compute loop


def _build_groups():
    # pairs of same-chunk tiles sharing one [128,512] PSUM bank; split at
    # bank boundaries so both reduce slots land in the same bank-pass
    groups = []
    k = 0
    for t in range(NCHUNK):
        ilist = list(range(2 * t + 1))
        while ilist:
            take = min(GMAX, len(ilist), TILES_PER_BANK - (k % TILES_PER_BANK))
            groups.append((t, ilist[:take]))
            ilist = ilist[take:]
            k += take
    return groups


GROUPS = _build_groups()

# Banks whose e-reduction runs on GPSIMD (partition-direction tensor_reduce,
# axis=C, one call per 64-partition half-group) instead of PE ones-matmuls.
# Offloads ~16% of the PE's reduce pass onto the otherwise idle Pool engine.
POOL_BANKS = ()
PO_ROW = 1024  # porow cols per half-group slot (GMAX * B_LOC)


def _bank_groups():
    out = {}
    k = 0
    for t, ilist in GROUPS:
        gsz = len(ilist)
        out.setdefault(k // TILES_PER_BANK, []).append((k, gsz))
        k += gsz
    return out


BANK_GROUPS = _bank_groups()
NPOOLB = max(1, len(POOL_BANKS))  # >=1 so the dram tensor stays valid
PO_NG = max([len(BANK_GROUPS[b]) for b in POOL_BANKS], default=1)  # groups per pool bank
PO_COLS = 2 * PO_NG * PO_ROW

WDMA_BATCH = 16  # stage-1 lhsT tiles per DMA


def host_prep(W: np.ndarray):
    """Build Wt3 [64, NTILES*128] f32, ONES [128, 512] bf16, PERM info."""
    # Wt2[d, p, e]
    Wt2 = np.ascontiguousarray(W.transpose(1, 0, 2))  # [64, 780, 64]
    pair_idx = -np.ones((F, F), dtype=np.int64)
    k = 0
    for i in range(F):
        for j in range(i + 1, F):
            pair_idx[i, j] = k
            k += 1
    Wt3 = np.zeros((D, NTILES * 128), dtype=np.float32)  # cast to bf16 at end
    # rows[k] = (origA or -1, origB) for tile k
    rows = []
    for k, (t, i) in enumerate(TILES):
        jA, jB = 2 * t, 2 * t + 1
        pA = pair_idx[i, jA] if i < jA else -1
        pB = pair_idx[i, jB]
        if pA >= 0:
            Wt3[:, k * 128 : k * 128 + 64] = Wt2[:, pA, :]
        Wt3[:, k * 128 + 64 : k * 128 + 128] = Wt2[:, pB, :]
        rows.append((pA, pB))
    # ones masks: ONES[:, q*32+m] — slot q (0..15): col 2q active for k<64,
    # col 2q+1 active for k>=64
    ones = np.zeros((128, 512), dtype=np.float32)
    for q in range(16):
        ones[0:64, q * 32 + 2 * q] = 1.0
        ones[64:128, q * 32 + 2 * q + 1] = 1.0
    ones = ones.astype(ml_dtypes.bfloat16)
    # out row of tile k: bank = k//64, s = k%64, g = s%4, q = s//4
    # rowA = bank*128 + 32*g + 2*q ; rowB = rowA + 1
    perm_src = np.zeros(P, dtype=np.int64)  # outT row for original pair p
    # pool-bank pairs come from poolout[pb, (gi*2+h)*PO_ROW + idx*B_LOC]
    pool_src = {}  # orig pair p -> (pb, col)
    kg = {}
    for b, glist in BANK_GROUPS.items():
        for gi, (k0, gsz) in enumerate(glist):
            for idx in range(gsz):
                kg[k0 + idx] = (b, gi, idx)
    for k, (pA, pB) in enumerate(rows):
        bank, s = divmod(k, TILES_PER_BANK)
        if bank in POOL_BANKS:
            pb = POOL_BANKS.index(bank)
            _, gi, idx = kg[k]
            if pA >= 0:
                pool_src[pA] = (pb, gi * 2 * PO_ROW + idx * B_LOC)
            pool_src[pB] = (pb, (gi * 2 + 1) * PO_ROW + idx * B_LOC)
            continue
        if REDUCE_COLTILE:
            g, q = s % 4, s // 4
            rowA = bank * BANK_ROWS + 32 * g + 2 * q
        else:
            rowA = bank * BANK_ROWS + 2 * s
        if pA >= 0:
            perm_src[pA] = rowA
        perm_src[pB] = rowA + 1
    return Wt3.astype(ml_dtypes.bfloat16), ones, perm_src, pool_src


def build_nc():
    nc = bacc.Bacc("TRN2", target_bir_lowering=False, debug=False)

    xtlo_dram = nc.dram_tensor(
        "xtlo", [64, F * B_LOC], BF16, kind="ExternalInput"
    ).ap()
    xtcb_dram = nc.dram_tensor(
        "xtcb", [128, NCHUNK * B_LOC], BF16, kind="ExternalInput"
    ).ap()
    wt_dram = nc.dram_tensor("Wt3", [D, NTILES * 128], BF16, kind="ExternalInput").ap()
    ones_dram = nc.dram_tensor("ones", [128, 512], BF16, kind="ExternalInput").ap()
    out_dram = nc.dram_tensor("outT", [OUT_ROWS, B_LOC], F32, kind="ExternalOutput").ap()
    po_dram = nc.dram_tensor(
        "poolout", [NPOOLB, PO_COLS], F32, kind="ExternalOutput"
    ).ap()

    with tile.TileContext(nc) as tc:
        with (
            tc.tile_pool(name="persist", bufs=1) as persist,
            tc.tile_pool(name="wpool", bufs=4) as wpool,
            tc.tile_pool(name="zpool", bufs=20) as zpool,
            tc.tile_pool(name="ybfpool", bufs=6) as ybfpool,
            tc.tile_pool(name="opool", bufs=2) as opool,
            tc.tile_pool(name="popool", bufs=2) as popool,
            tc.tile_pool(name="ypsum", bufs=3, space=bass.MemorySpace.PSUM) as ypsum,
            tc.tile_pool(name="rpsum", bufs=1, space=bass.MemorySpace.PSUM) as rpsum,
        ):
            ones = persist.tile([128, 512], BF16, tag="ones")

            # XTC[(f%2)*64 + d, t*256 + m*128 + b]  (f = 2t + f%2) and the
            # low-half layout (all fields at partitions 0-63) are both
            # pre-transposed on the host and DMA'd directly.
            xtlo = persist.tile([64, F * B_LOC], BF16, tag="xtlo")
            xtcb = persist.tile([128, NCHUNK * B_LOC], BF16, tag="xtcb")

            # W batches are DMA'd on the SP queue; x layouts ride the ACT
            # HWDGE queue so both descriptor-gen streams overlap. Pieces are
            # issued smallest-first so early tiles unblock within ~2us.
            wtiles = []

            def w_dma(bi):
                kt0 = bi * WDMA_BATCH
                nw = min(WDMA_BATCH, NTILES - kt0)
                wt = wpool.tile([64, WDMA_BATCH * 128], BF16, tag="w")
                if bi == 0:
                    # two half-DMAs: tiles 0-7 land ~0.4us sooner
                    nc.sync.dma_start(out=wt[:, : 8 * 128], in_=wt_dram[:, : 8 * 128])
                    nc.sync.dma_start(
                        out=wt[:, 8 * 128 : nw * 128],
                        in_=wt_dram[:, 8 * 128 : nw * 128],
                    )
                else:
                    nc.sync.dma_start(
                        out=wt[:, : nw * 128],
                        in_=wt_dram[:, kt0 * 128 : (kt0 + nw) * 128],
                    )
                wtiles.append(wt)

            def x_dma(piece, eng):
                kind, lo, hi = piece
                src, dst = {
                    "xtlo": (xtlo_dram, xtlo),
                    "xtcb": (xtcb_dram, xtcb),
                }[kind]
                eng.dma_start(
                    out=dst[:, lo * B_LOC : hi * B_LOC],
                    in_=src[:, lo * B_LOC : hi * B_LOC],
                )

            # First pieces on the ACT HWDGE queue (3 gens before the first
            # evict dispatch); the tail rides Pool SWDGE, which is idle until
            # the out-DMAs begin.
            w_dma(0)
            x_dma(("xtlo",) + XTLO_PIECES[0], nc.scalar)
            nc.gpsimd.dma_start(out=ones[:], in_=ones_dram[:])
            x_dma(("xtcb",) + XCHUNK_PIECES[0], nc.gpsimd)
            x_dma(("xtlo",) + XTLO_PIECES[1], nc.scalar)
            w_dma(1)
            x_dma(("xtcb",) + XCHUNK_PIECES[1], nc.gpsimd)
            x_dma(("xtlo",) + XTLO_PIECES[2], nc.scalar)
            w_dma(2)
            x_dma(("xtcb",) + XCHUNK_PIECES[2], nc.gpsimd)
            x_dma(("xtlo",) + XTLO_PIECES[3], nc.gpsimd)
            w_dma(3)
            x_dma(("xtcb",) + XCHUNK_PIECES[3], nc.gpsimd)
            x_dma(("xtlo",) + XTLO_PIECES[4], nc.gpsimd)

            # one persistent 2-bank reduce tile, 4 slots of [0:32, 256]:
            # bank b -> slot ((b>>1)&1)*2 + (b&1); pairs of banks evict in a
            # single [32, 512] ACT copy + one DMA (frees the DVE entirely)
            rball = rpsum.tile([128, 4 * B_LOC], F32, tag="rball", name="rball")
            k = 0
            nquad = 0
            pending = []

            porows = [None]

            def emit_reduce(z, k0, gsz):
                bank0 = k0 // TILES_PER_BANK
                if bank0 in POOL_BANKS:
                    # GPSIMD partition reduce: one [64, gsz*256] half-group
                    # per call -> [1, gsz*256] pair-rows on partition 0
                    glist = BANK_GROUPS[bank0]
                    gi = [g[0] for g in glist].index(k0)
                    if gi == 0:
                        porows[0] = popool.tile([1, PO_COLS], F32, tag="po", name="po")
                    po = porows[0]
                    for h in range(2):
                        c0 = (gi * 2 + h) * PO_ROW
                        nc.gpsimd.tensor_reduce(
                            out=po[:, c0 : c0 + gsz * B_LOC],
                            in_=z[h * 64 : (h + 1) * 64, : gsz * B_LOC],
                            axis=mybir.AxisListType.C,
                            op=mybir.AluOpType.add,
                        )
                    if gi == len(glist) - 1:
                        pb = POOL_BANKS.index(bank0)
                        nc.sync.dma_start(
                            out=po_dram[pb : pb + 1, :], in_=po[:]
                        )
                    return
                # accumulate into reduce bank via ones-mask matmuls
                for idx in range(gsz):
                    kt = k0 + idx
                    bank, s = divmod(kt, TILES_PER_BANK)
                    q = s
                    slot = ((bank >> 1) & 1) * 2 + (bank & 1)
                    rb = rball[:, slot * B_LOC : (slot + 1) * B_LOC]
                    last_in_bank = (s == TILES_PER_BANK - 1) or (kt == NTILES - 1)
                    nc.tensor.matmul(
                        rb[0:32, :],
                        ones[:, q * 32 : (q + 1) * 32],
                        z[:, idx * B_LOC : (idx + 1) * B_LOC],
                        start=(s == 0),
                        stop=last_in_bank,
                        tile_position=(0, 0),
                        skip_group_check=True,
                    )
                    if last_in_bank and bank % 2 == 1:
                        ps = (slot // 2) * 2
                        ob = opool.tile([BANK_ROWS, 2 * B_LOC], F32, tag="ob")
                        nc.scalar.copy(
                            out=ob[:],
                            in_=rball[0:BANK_ROWS, ps * B_LOC : (ps + 2) * B_LOC],
                        )
                        # SWDGE (Pool) out-DMA keeps the SP queue free for W
                        # batches; the last pair rides SP (idle by then) to
                        # shorten the drain tail.
                        oeng = nc.sync if bank >= NBANKS - 3 else nc.gpsimd
                        oeng.dma_start(
                            out=out_dram[
                                (bank - 1) * BANK_ROWS : (bank + 1) * BANK_ROWS, :
                            ].rearrange("(s p) c -> p s c", s=2),
                            in_=ob[:].rearrange("p (s c) -> p s c", s=2),
                        )
                    elif last_in_bank and kt == NTILES - 1:
                        ob = opool.tile([BANK_ROWS, B_LOC], F32, tag="ob")
                        nc.scalar.copy(out=ob[:], in_=rb[0:BANK_ROWS, :])
                        nc.sync.dma_start(
                            out=out_dram[
                                bank * BANK_ROWS : (bank + 1) * BANK_ROWS, :
                            ],
                            in_=ob[:],
                        )

            for gidx, (t, ilist) in enumerate(GROUPS):
                gsz = len(ilist)
                # stage 1: Y[(p, e), b] = Wtile.T @ xT_i — gsz tiles share
                # one PSUM bank (disjoint column halves)
                y = ypsum.tile([128, GMAX * B_LOC], F32, tag="y")
                for idx, i in enumerate(ilist):
                    kt = k + idx
                    if kt % WDMA_BATCH == 0:
                        bi = kt // WDMA_BATCH + 2  # two batches of DMA lead
                        if WDMA_PREFETCH <= bi < (NTILES + WDMA_BATCH - 1) // WDMA_BATCH:
                            w_dma(bi)
                    kk = kt % WDMA_BATCH
                    wchunk = wtiles[kt // WDMA_BATCH]
                    nc.tensor.matmul(
                        y[:, idx * B_LOC : (idx + 1) * B_LOC],
                        wchunk[:, kk * 128 : (kk + 1) * 128],
                        xtlo[:, i * B_LOC : (i + 1) * B_LOC],
                        start=True,
                        stop=True,
                    )

                # stage 2: z = Y * xT[j-fields chunk t]  (bf16 out), one TT
                # per group with stride-0 broadcast of the xtc chunk.
                # A fraction of pair-groups takes the ACT-evict + GPSIMD
                # multiply path to unload the DVE.
                z = zpool.tile([128, GMAX * B_LOC], BF16, tag="z")
                if True:
                    if gsz > 1:
                        nquad += 1
                    act_path = gsz == 1 or nquad % 4 != 1
                    if act_path:
                        # ACT evicts PSUM -> bf16, DVE multiplies at 2x
                        ybf = ybfpool.tile([128, GMAX * B_LOC], BF16, tag="ybf")
                        nc.scalar.copy(
                            out=ybf[:, : gsz * B_LOC], in_=y[:, : gsz * B_LOC]
                        )
                        in1 = xtcb[
                            :, None, t * B_LOC : (t + 1) * B_LOC
                        ].to_broadcast([128, gsz, B_LOC])
                        nc.vector.tensor_tensor(
                            z[:, : gsz * B_LOC].rearrange(
                                "p (n b) -> p n b", n=gsz
                            ),
                            ybf[:, : gsz * B_LOC].rearrange(
                                "p (n b) -> p n b", n=gsz
                            ),
                            in1,
                            mybir.AluOpType.mult,
                        )
                    else:
                        # every 9th quad drains fused on the DVE (PSUM f32
                        # multiply at 1x) to keep ACT under the PE roofline
                        in1 = xtcb[
                            :, None, t * B_LOC : (t + 1) * B_LOC
                        ].to_broadcast([128, gsz, B_LOC])
                        nc.vector.tensor_tensor(
                            z[:, : gsz * B_LOC].rearrange(
                                "p (n b) -> p n b", n=gsz
                            ),
                            y[:, : gsz * B_LOC].rearrange(
                                "p (n b) -> p n b", n=gsz
                            ),
                            in1,
                            mybir.AluOpType.mult,
                        )
                else:
                    nc.vector.tensor_tensor(
                        z[:, :B_LOC],
                        y[:, :B_LOC],
                        xtcb[:, t * B_LOC : (t + 1) * B_LOC],
                        mybir.AluOpType.mult,
                    )

                # stage 3 is software-pipelined: queue this group's reduce
                # and emit the one from REDUCE_DELAY groups ago, so the PE's
                # in-order queue never waits on the just-issued ACT->DVE
                # multiply chain.
                pending.append((z, k, gsz))
                if len(pending) > REDUCE_DELAY:
                    emit_reduce(*pending.pop(0))
                # taper the reduce lag over the final stretch so the last
                # banks' reduces interleave with stage-1 instead of running
                # as a pure PE flush after it
                if gidx >= len(GROUPS) - 3 * REDUCE_DELAY and len(pending) > 1:
                    emit_reduce(*pending.pop(0))
                k += gsz

            while pending:
                emit_reduce(*pending.pop(0))

    nc.compile()
    return nc


_NC = None


def kernel(x: np.ndarray, W: np.ndarray) -> np.ndarray:
    global _NC
    x = np.ascontiguousarray(np.asarray(x, dtype=np.float32))
    W = np.ascontiguousarray(np.asarray(W, dtype=np.float32))
    assert x.shape == (B, F, D) and W.shape == (P, D, D)

    Wt3, ones, perm_src, pool_src = host_prep(W)

    if _NC is None:
        _NC = build_nc()

    in_maps = []
    for c in range(NCORES):
        xs = x[c * B_LOC : (c + 1) * B_LOC]  # [256, 40, 64]
        v = xs.transpose(1, 2, 0).reshape(NCHUNK, 2, D, B_LOC)
        xtc = np.ascontiguousarray(
            v.transpose(1, 2, 0, 3).reshape(128, NCHUNK * B_LOC)
        )
        xtlo = np.ascontiguousarray(
            xs.transpose(2, 1, 0).reshape(D, F * B_LOC)
        ).astype(ml_dtypes.bfloat16)
        xtcb = xtc.astype(ml_dtypes.bfloat16)
        in_maps.append(
            {"xtcb": xtcb, "xtlo": xtlo, "Wt3": Wt3, "ones": ones}
        )
    res = run_bass_kernel_spmd(_NC, in_maps, core_ids=list(range(NCORES)))
    out = np.empty((B, P), dtype=np.float32)
    for c in range(NCORES):
        outT = res.results[c]["outT"]  # [OUT_ROWS, B_LOC]
        out[c * B_LOC : (c + 1) * B_LOC, :] = outT[perm_src, :].T
        po = res.results[c]["poolout"]  # [NPOOLB, PO_COLS]
        for p, (pb, col) in pool_src.items():
            out[c * B_LOC : (c + 1) * B_LOC, p] = po[pb, col : col + B_LOC]
    return out

